# revision 35
# baseline (speedup 1.0000x reference)
"""Trainium2 Bass kernel for MixtralBlockSparseTop2MLP grouped-GEMM MoE.

Problem: 4096 rows (sorted by expert), 8 experts, hidden=1024, ffn=3584.
  out[r] = silu(x[r] @ W1g[e(r)]) * (x[r] @ W1u[e(r)]) @ W2[e(r)]

Sharding: tensor-parallel over the ffn dimension. Each of the 8 cores gets
a 448-channel slice of every expert's gate/up/down weights and computes a
partial output for ALL 4096 rows; the host sums the 8 partials. All cores
run the identical program (segment structure baked from rows_for_experts at
call time), so one SPMD NEFF serves all 8 cores with per-core weight data.

Compute dtype: bf16 matmul inputs with fp32 PSUM accumulation (fp32 matmul
is 4x slower on TRN2). Measured end-to-end rel err vs fp32 reference ~4e-3.

Schedule notes (v5):
- Weights alone on the sync queue (one single-shot DMA per tensor: each
  dma_start costs its issuing engine ~0.6-1.6us of descriptor generation,
  so weight prefetch is issue-rate-limited with per-k slicing), x on
  gpsimd, output stores alternate scalar/gpsimd.
- x is packed TIGHT per chunk (flat [P, sum(KO*nch)]) so partial chunks
  don't load 512-row padding; the first ~50us is DMA-bandwidth-bound, so
  early bytes are the scarcest resource.
- Segments ordered: two biggest experts first (max early PE work per DMA
  byte), small experts interleaved later where DMA has slack, and a split
  expert last so the final chunk (its tail) has a small final store.
- First chunk runs its 7 gemm1 PSUM groups k-major (6 hps banks + 1
  borrowed ops bank) so the PE issues 7 matmuls per arriving (x, w1)
  k-slice pair through the DMA prologue and warms the HAM clock gate once.
- Oversize segments split 128-aligned near-balanced (527 -> 384+143, not
  512+15) so no chunk's matmuls fall under the weight-load issue floor.
"""

import os
import sys

sys.path.insert(0, "/opt/trn_rl_repo")

import numpy as np
import ml_dtypes

E, R, H, F = 8, 1024 * 4, 1024, 3584
FC = F // 8          # 448 ffn channels per core
FCP = 512            # per-core ffn padded to 4 k-tiles of 128 for gemm2
NCH = 512            # row-chunk (gemm1 moving free dim; one PSUM bank per unit)
P = 128

BF16 = ml_dtypes.bfloat16

# test.py introspection: last BassKernelResults from run_bass_kernel_spmd
LAST_RESULT = None

_PROGRAM_CACHE = {}


def _split(n):
    """Split a segment of n rows into <=512 chunks, 128-aligned and near-
    balanced so no chunk is tiny (tiny chunks pay full weight-load time for
    little streaming work)."""
    if n <= NCH:
        return [n]
    k = -(-n // NCH)
    p = min(NCH, -(-(-(-n // k)) // P) * P)
    parts = [p] * (k - 1) + [n - p * (k - 1)]
    assert all(1 <= x <= NCH for x in parts) and sum(parts) == n, (n, parts)
    return parts


def _segments(rows_for_experts):
    """[(expert, row_start, n_rows)] in processing order."""
    segs = []
    r0 = 0
    for e in range(E):
        n = int(rows_for_experts[e])
        if n > 0:
            segs.append((e, r0, n))
        r0 += n
    bigs = sorted([s for s in segs if s[2] >= 256], key=lambda s: -s[2])
    smalls = sorted([s for s in segs if s[2] < 256], key=lambda s: s[2])
    # two biggest first (the startup window is DMA-bound: maximize early PE
    # work per weight byte), then interleave smalls into the remaining bigs.
    order = bigs[:2]
    bi = si = 0
    rest = bigs[2:]
    while bi < len(rest) or si < len(smalls):
        if bi < len(rest):
            order.append(rest[bi])
            bi += 1
        if si < len(smalls):
            order.append(smalls[si])
            si += 1
    # end on an expert that splits, so the final chunk (tail of its split)
    # is small: its store is the only work left after the last matmul.
    if len(order) >= 2 and order[-1][2] <= NCH:
        for j in range(len(order) - 2, 0, -1):
            if order[j][2] > NCH:
                order.append(order.pop(j))
                break
    return order


def _chunk_list(segments):
    """[(expert, row_start, nch)] in program iteration order."""
    out = []
    for (e, r0, n_e) in segments:
        c0 = 0
        for part in _split(n_e):
            out.append((e, r0 + c0, part))
            c0 += part
    return out


def _build_program(segments, act_mode="silu"):
    import concourse.mybir as mybir
    import concourse.tile as tile
    from concourse import bacc

    dt = mybir.dt
    nc = bacc.Bacc(None, target_bir_lowering=False, debug=False)

    chunks = _chunk_list(segments)
    n_chunks = len(chunks)
    offs = []
    tot = 0
    for (_, _, nch) in chunks:
        offs.append(tot)
        tot += (H // P) * nch

    xT = nc.declare_dram_parameter("xT", [P, tot], dt.bfloat16, isOutput=False)
    w1 = nc.declare_dram_parameter("w1c", [E, P, H // P, 2 * FC], dt.bfloat16, isOutput=False)
    w2 = nc.declare_dram_parameter("w2c", [E, P, FCP // P, H], dt.bfloat16, isOutput=False)
    # transposed tight output: per chunk, 8 h-tiles of [P, nch] at column
    # offset offs[ci] (same offsets as xT — both are 8*nch cols per chunk)
    outp = nc.declare_dram_parameter("outp", [P, tot], dt.bfloat16, isOutput=True)

    KO = H // P        # 8 k-tiles for gemm1
    K2 = FCP // P      # 4 k-tiles for gemm2
    silu = mybir.ActivationFunctionType.Silu
    sigmoid = mybir.ActivationFunctionType.Sigmoid

    with tile.TileContext(nc) as tc:
        with (
            tc.tile_pool(name="w1p", bufs=3) as w1p,
            tc.tile_pool(name="w2p", bufs=3) as w2p,
            tc.tile_pool(name="xp", bufs=4) as xp,
            tc.tile_pool(name="apool", bufs=2) as apool,
            tc.tile_pool(name="a3pool", bufs=1) as a3pool,
            tc.tile_pool(name="opool", bufs=4) as opool,
            tc.tile_pool(name="hps", bufs=6, space="PSUM") as hps,
            tc.tile_pool(name="ops", bufs=2, space="PSUM") as ops,
        ):
            # a tile 3 holds only unit 6 in rows 0:64; rows 64:128 must be
            # exact zeros (they multiply the zero-padded w2 rows). Two
            # persistent ping-pong tiles, zeroed once.
            a3_tiles = [
                a3pool.tile([P, NCH], dt.bfloat16, tag=f"a3_{i}", name=f"a3_{i}")
                for i in range(2)
            ]
            for t3 in a3_tiles:
                nc.vector.memset(t3[:], 0.0)

            def act_mul(u, hu_ps, a_tiles, nch):
                """silu(gate_u) * up_u from a closed PSUM group into the
                packed a k-tiles (gate in psum rows 0:64, up in 64:128)."""
                stmp = apool.tile([64, NCH], dt.bfloat16, tag="stmp", name="stmp")
                if act_mode == "silu":
                    nc.scalar.activation(stmp[:, :nch], hu_ps[0:64, :nch], silu)
                else:  # silu(g) = g * sigmoid(g); CoreSim lacks Silu
                    nc.scalar.activation(stmp[:, :nch], hu_ps[0:64, :nch], sigmoid)
                    nc.vector.tensor_mul(
                        stmp[:, :nch], stmp[:, :nch], hu_ps[0:64, :nch]
                    )
                lo = 64 * (u % 2)
                nc.vector.tensor_mul(
                    a_tiles[u // 2][lo : lo + 64, :nch],
                    stmp[:, :nch],
                    hu_ps[64:128, :nch],
                )

            chunk_idx = 0
            first = True
            pending_gemm2 = None
            for (e, r0, n_e) in segments:
                w1sb = w1p.tile([P, KO, 2 * FC], dt.bfloat16, tag="w1sb")
                w2sb = w2p.tile([P, K2, H], dt.bfloat16, tag="w2sb")
                if first:
                    # prologue: paired k-slices, x alternating scalar/gpsimd
                    # parallel to w1 pairs on sync — few descriptors (each
                    # costs ~1us of engine issue time) but still progressive
                    # arrival for the k-major loop below.
                    nch0 = _split(n_e)[0]
                    xsb0 = xp.tile([P, KO * NCH], dt.bfloat16, tag="xsb")
                    # paired k-slices, x alternating scalar/gpsimd parallel
                    # to w1 pairs on sync — few descriptors (each costs ~1us
                    # of engine issue time) but still progressive arrival
                    # for the k-major loop below.
                    for i, (ka, kb) in enumerate([(0, 2), (2, 4), (4, 6), (6, 8)]):
                        xq = nc.scalar if i % 2 == 0 else nc.gpsimd
                        xq.dma_start(
                            xsb0[:, ka * nch0 : kb * nch0],
                            xT[:, ka * nch0 : kb * nch0],
                        )
                        nc.sync.dma_start(
                            w1sb[:, ka:kb, :], w1[e, :, ka:kb, :]
                        )
                else:
                    # single-shot per expert: weight prefetch is limited by
                    # DMA issue rate, not bandwidth — 1 descriptor, not 8.
                    xsb0 = None
                    nc.sync.dma_start(w1sb[:], w1[e])
                nc.sync.dma_start(w2sb[:], w2[e])

                for part in _split(n_e):
                    nch = part

                    if xsb0 is not None:
                        xsb, xsb0 = xsb0, None
                    else:
                        xsb = xp.tile([P, KO * NCH], dt.bfloat16, tag="xsb")
                        nc.gpsimd.dma_start(
                            xsb[:, : KO * nch],
                            xT[:, offs[chunk_idx] : offs[chunk_idx] + KO * nch],
                        )

                    def xk(k, nch=nch, xsb=xsb):
                        return xsb[:, k * nch : k * nch + nch]

                    a_tiles = [
                        apool.tile([P, NCH], dt.bfloat16, tag=f"a{j}", name=f"a{j}")
                        if j < 3
                        else a3_tiles[chunk_idx % 2]
                        for j in range(4)
                    ]

                    if first:
                        # k-major: all 7 units accumulate per arriving
                        # (x[k], w1[k]) slice pair — 7 matmuls per DMA step
                        # keeps the PE busy through the cold prologue. Units
                        # 0..5 in hps banks, unit 6 borrows an ops bank
                        # (gemm2 doesn't need it until a chunk later).
                        first = False
                        hu_tiles = [
                            hps.tile([P, NCH], dt.float32, tag="h", name=f"h{u}")
                            for u in range(6)
                        ] + [ops.tile([P, NCH], dt.float32, tag="o", name="h6")]
                        for k in range(KO):
                            for u in range(7):
                                nc.tensor.matmul(
                                    hu_tiles[u][:, :nch],
                                    w1sb[:, k, P * u : P * u + P],
                                    xk(k),
                                    start=(k == 0),
                                    stop=(k == KO - 1),
                                )
                        for u in range(7):
                            act_mul(u, hu_tiles[u], a_tiles, nch)
                    else:
                        # u-major: one PSUM group at a time; silu/mul of
                        # group u overlaps the matmuls of group u+1.
                        for u in (6, 0, 1, 2, 3, 4, 5):
                            hu_ps = hps.tile([P, NCH], dt.float32, tag="h", name=f"h{u}")
                            for k in range(KO):
                                nc.tensor.matmul(
                                    hu_ps[:, :nch],
                                    w1sb[:, k, P * u : P * u + P],
                                    xk(k),
                                    start=(k == 0),
                                    stop=(k == KO - 1),
                                )
                            act_mul(u, hu_ps, a_tiles, nch)

                    # gemm2, TRANSPOSED (emitted one chunk behind gemm1 so
                    # the PE never waits on this chunk's silu/mul chain):
                    # out.T[h, j] = sum_f w2[f, h] * a[f, j]. The moving
                    # operand is the chunk's rows, so a partial chunk
                    # streams exactly nch columns per (h, k) instead of
                    # full 512-col H-slices per 128-row s-tile — ragged
                    # chunks stop paying for padding (FWL keeps the weight
                    # load ~53ns, under the smallest chunk's stream time).
                    def gemm2(
                        ci=chunk_idx, nch=nch, a_tiles=a_tiles, w2sb=w2sb
                    ):
                        osb = opool.tile(
                            [P, KO * NCH], dt.bfloat16, tag="osb", name="osb"
                        )
                        last = ci == n_chunks - 1
                        for h in range(H // P):
                            ot = ops.tile(
                                [P, NCH], dt.float32, tag="o", name=f"o{h}"
                            )
                            for k in range(K2):
                                nc.tensor.matmul(
                                    ot[:, :nch],
                                    w2sb[:, k, P * h : P * h + P],
                                    a_tiles[k][:, :nch],
                                    start=(k == 0),
                                    stop=(k == K2 - 1),
                                )
                            nc.vector.tensor_copy(
                                osb[:, h * nch : h * nch + nch], ot[:, :nch]
                            )
                            if last and h == 3:
                                # final chunk: first half drains during the
                                # remaining h-tiles' matmuls; only the second
                                # half trails the last matmul.
                                nc.gpsimd.dma_start(
                                    outp[:, offs[ci] : offs[ci] + 4 * nch],
                                    osb[:, : 4 * nch],
                                )
                        # one tight contiguous store per chunk (transposed
                        # [h, j] layout; the host transposes back). Alternate
                        # queues to spread descriptor-generation cost.
                        if last:
                            nc.scalar.dma_start(
                                outp[:, offs[ci] + 4 * nch : offs[ci] + KO * nch],
                                osb[:, 4 * nch : KO * nch],
                            )
                        else:
                            sq = nc.scalar if ci % 2 == 0 else nc.gpsimd
                            sq.dma_start(
                                outp[:, offs[ci] : offs[ci] + KO * nch],
                                osb[:, : KO * nch],
                            )

                    if pending_gemm2 is not None:
                        pending_gemm2()
                    pending_gemm2 = gemm2
                    chunk_idx += 1
            pending_gemm2()

    nc.compile()
    return nc


def _prepare_inputs(hidden_states, w1, w2, chunks):
    """Host-side shard/layout/cast. Returns (xT, [w1c per core], [w2c per core])."""
    x = np.asarray(hidden_states, dtype=np.float32)
    w1 = np.asarray(w1, dtype=np.float32)
    w2 = np.asarray(w2, dtype=np.float32)

    xb = x.astype(BF16)          # [R, H]
    w1b = w1.astype(BF16)        # [E, H, 2F]
    w2b = w2.astype(BF16)        # [E, F, H]

    # tight chunk-major x: for chunk c at column offset off_c,
    # xT[p, off_c + k*nch + j] = x[r_c + j, 128*k + p]
    tot = sum((H // P) * nch for (_, _, nch) in chunks)
    xT = np.empty((P, tot), dtype=BF16)
    off = 0
    for (_, r, nch) in chunks:
        blk = xb[r : r + nch, :].T.reshape(H // P, P, nch)  # [k, p, j]
        xT[:, off : off + (H // P) * nch] = np.ascontiguousarray(
            blk.transpose(1, 0, 2)
        ).reshape(P, (H // P) * nch)
        off += (H // P) * nch

    w1cs, w2cs = [], []
    for c in range(8):
        gate = w1b[:, :, c * FC : (c + 1) * FC]
        up = w1b[:, :, F + c * FC : F + (c + 1) * FC]
        # interleave 64-channel blocks: [G0|U0|G1|U1|...|G6|U6] so each
        # 128-column m-slice u packs gate_u in psum partitions 0:64 and
        # up_u in 64:128.
        w1cat = np.ascontiguousarray(
            np.stack(
                [gate.reshape(E, H, FC // 64, 64), up.reshape(E, H, FC // 64, 64)],
                axis=3,
            ).reshape(E, H, 2 * FC)
        )
        w1c = np.ascontiguousarray(
            w1cat.reshape(E, H // P, P, 2 * FC).transpose(0, 2, 1, 3)
        )
        w2pad = np.zeros((E, FCP, H), dtype=BF16)
        w2pad[:, :FC, :] = w2b[:, c * FC : (c + 1) * FC, :]
        w2c = np.ascontiguousarray(
            w2pad.reshape(E, FCP // P, P, H).transpose(0, 2, 1, 3)
        )
        w1cs.append(w1c)
        w2cs.append(w2c)
    return xT, w1cs, w2cs


def kernel(hidden_states, w1, w2, rows_for_experts):
    global LAST_RESULT
    from concourse.bass_utils import run_bass_kernel_spmd

    segs = _segments(np.asarray(rows_for_experts))
    if not segs:
        return np.zeros((R, H), dtype=np.float32)
    key = tuple(segs)
    nc = _PROGRAM_CACHE.get(key)
    if nc is None:
        nc = _build_program(segs)
        _PROGRAM_CACHE[key] = nc

    chunks = _chunk_list(segs)
    xT, w1cs, w2cs = _prepare_inputs(hidden_states, w1, w2, chunks)
    in_maps = [
        {"xT": xT, "w1c": w1cs[c], "w2c": w2cs[c]} for c in range(8)
    ]
    res = run_bass_kernel_spmd(nc, in_maps, core_ids=list(range(8)))
    LAST_RESULT = res

    acc = np.zeros((R, H), dtype=np.float32)
    for c in range(8):
        blocks = res.results[c]["outp"]  # [P, sum(8*nch)] bf16, transposed
        off = 0
        for (_, r, nch) in chunks:
            blk = blocks[:, off : off + (H // P) * nch].reshape(P, H // P, nch)
            rowsmaj = blk.transpose(1, 0, 2).reshape(H, nch).T
            acc[r : r + nch] += rowsmaj.astype(np.float32)
            off += (H // P) * nch
    return acc


# revision 36
# speedup vs baseline: 1.1842x; 1.1842x over previous
"""Trainium2 Bass kernel for MixtralBlockSparseTop2MLP grouped-GEMM MoE.

Problem: 4096 rows (sorted by expert), 8 experts, hidden=1024, ffn=3584.
  out[r] = silu(x[r] @ W1g[e(r)]) * (x[r] @ W1u[e(r)]) @ W2[e(r)]

Sharding: tensor-parallel over the ffn dimension. Each of the 8 cores gets
a 448-channel slice of every expert's gate/up/down weights and computes a
partial output for ALL 4096 rows; the host sums the 8 partials. All cores
run the identical program (segment structure baked from rows_for_experts at
call time), so one SPMD NEFF serves all 8 cores with per-core weight data.

Compute dtype: bf16 matmul inputs with fp32 PSUM accumulation (fp32 matmul
is 4x slower on TRN2). Measured end-to-end rel err vs fp32 reference ~4e-3.

Schedule notes (v5):
- Weights alone on the sync queue (one single-shot DMA per tensor: each
  dma_start costs its issuing engine ~0.6-1.6us of descriptor generation,
  so weight prefetch is issue-rate-limited with per-k slicing), x on
  gpsimd, output stores alternate scalar/gpsimd.
- x is packed TIGHT per chunk (flat [P, sum(KO*nch)]) so partial chunks
  don't load 512-row padding; the first ~50us is DMA-bandwidth-bound, so
  early bytes are the scarcest resource.
- Segments ordered: two biggest experts first (max early PE work per DMA
  byte), small experts interleaved later where DMA has slack, and a split
  expert last so the final chunk (its tail) has a small final store.
- First chunk runs its 7 gemm1 PSUM groups k-major (6 hps banks + 1
  borrowed ops bank) so the PE issues 7 matmuls per arriving (x, w1)
  k-slice pair through the DMA prologue and warms the HAM clock gate once.
- Oversize segments split 128-aligned near-balanced (527 -> 384+143, not
  512+15) so no chunk's matmuls fall under the weight-load issue floor.
"""

import os
import sys

sys.path.insert(0, "/opt/trn_rl_repo")

import numpy as np
import ml_dtypes

E, R, H, F = 8, 1024 * 4, 1024, 3584
FC = F // 8          # 448 ffn channels per core
FCP = 512            # per-core ffn padded to 4 k-tiles of 128 for gemm2
NCH = 512            # row-chunk (gemm1 moving free dim; one PSUM bank per unit)
P = 128

BF16 = ml_dtypes.bfloat16

# test.py introspection: last BassKernelResults from run_bass_kernel_spmd
LAST_RESULT = None

_PROGRAM_CACHE = {}


def _split(n):
    """Split a segment of n rows into <=512 chunks, 128-aligned and near-
    balanced so no chunk is tiny (tiny chunks pay full weight-load time for
    little streaming work)."""
    if n <= NCH:
        return [n]
    k = -(-n // NCH)
    p = min(NCH, -(-(-(-n // k)) // P) * P)
    parts = [p] * (k - 1) + [n - p * (k - 1)]
    assert all(1 <= x <= NCH for x in parts) and sum(parts) == n, (n, parts)
    return parts


def _segments(rows_for_experts):
    """[(expert, row_start, n_rows)] in processing order."""
    segs = []
    r0 = 0
    for e in range(E):
        n = int(rows_for_experts[e])
        if n > 0:
            segs.append((e, r0, n))
        r0 += n
    bigs = sorted([s for s in segs if s[2] >= 256], key=lambda s: -s[2])
    smalls = sorted([s for s in segs if s[2] < 256], key=lambda s: s[2])
    # two biggest first (the startup window is DMA-bound: maximize early PE
    # work per weight byte), then interleave smalls into the remaining bigs.
    order = bigs[:2]
    bi = si = 0
    rest = bigs[2:]
    while bi < len(rest) or si < len(smalls):
        if bi < len(rest):
            order.append(rest[bi])
            bi += 1
        if si < len(smalls):
            order.append(smalls[si])
            si += 1
    # end on an expert that splits, so the final chunk (tail of its split)
    # is small: its store is the only work left after the last matmul.
    if len(order) >= 2 and order[-1][2] <= NCH:
        for j in range(len(order) - 2, 0, -1):
            if order[j][2] > NCH:
                order.append(order.pop(j))
                break
    return order


def _chunk_list(segments):
    """[(expert, row_start, nch)] in program iteration order."""
    out = []
    for (e, r0, n_e) in segments:
        c0 = 0
        for part in _split(n_e):
            out.append((e, r0 + c0, part))
            c0 += part
    return out


def _build_program(segments, act_mode="silu"):
    import concourse.mybir as mybir
    import concourse.tile as tile
    from concourse import bacc

    dt = mybir.dt
    nc = bacc.Bacc(None, target_bir_lowering=False, debug=False)

    chunks = _chunk_list(segments)
    n_chunks = len(chunks)
    offs = []
    tot = 0
    for (_, _, nch) in chunks:
        offs.append(tot)
        tot += (H // P) * nch

    xT = nc.declare_dram_parameter("xT", [P, tot], dt.bfloat16, isOutput=False)
    w1 = nc.declare_dram_parameter("w1c", [E, P, H // P, 2 * FC], dt.bfloat16, isOutput=False)
    w2 = nc.declare_dram_parameter("w2c", [E, P, FCP // P, H], dt.bfloat16, isOutput=False)
    # transposed tight output: per chunk, 8 h-tiles of [P, nch] at column
    # offset offs[ci] (same offsets as xT — both are 8*nch cols per chunk)
    outp = nc.declare_dram_parameter("outp", [P, tot], dt.bfloat16, isOutput=True)

    KO = H // P        # 8 k-tiles for gemm1
    K2 = FCP // P      # 4 k-tiles for gemm2
    silu = mybir.ActivationFunctionType.Silu
    sigmoid = mybir.ActivationFunctionType.Sigmoid

    with tile.TileContext(nc) as tc:
        with (
            tc.tile_pool(name="w1p", bufs=3) as w1p,
            tc.tile_pool(name="w2p", bufs=3) as w2p,
            tc.tile_pool(name="xp", bufs=4) as xp,
            tc.tile_pool(name="apool", bufs=2) as apool,
            tc.tile_pool(name="a3pool", bufs=1) as a3pool,
            tc.tile_pool(name="opool", bufs=4) as opool,
            tc.tile_pool(name="hps", bufs=6, space="PSUM") as hps,
            tc.tile_pool(name="ops", bufs=2, space="PSUM") as ops,
        ):
            # a tile 3 holds only unit 6 in rows 0:64; rows 64:128 must be
            # exact zeros (they multiply the zero-padded w2 rows). Two
            # persistent ping-pong tiles, zeroed once.
            a3_tiles = [
                a3pool.tile([P, NCH], dt.bfloat16, tag=f"a3_{i}", name=f"a3_{i}")
                for i in range(2)
            ]
            for t3 in a3_tiles:
                nc.vector.memset(t3[:], 0.0)

            def act_mul(u, hu_ps, a_tiles, nch):
                """silu(gate_u) * up_u from a closed PSUM group into the
                packed a k-tiles (gate in psum rows 0:64, up in 64:128)."""
                stmp = apool.tile([64, NCH], dt.bfloat16, tag="stmp", name="stmp")
                if act_mode == "silu":
                    nc.scalar.activation(stmp[:, :nch], hu_ps[0:64, :nch], silu)
                else:  # silu(g) = g * sigmoid(g); CoreSim lacks Silu
                    nc.scalar.activation(stmp[:, :nch], hu_ps[0:64, :nch], sigmoid)
                    nc.vector.tensor_mul(
                        stmp[:, :nch], stmp[:, :nch], hu_ps[0:64, :nch]
                    )
                lo = 64 * (u % 2)
                nc.vector.tensor_mul(
                    a_tiles[u // 2][lo : lo + 64, :nch],
                    stmp[:, :nch],
                    hu_ps[64:128, :nch],
                )

            chunk_idx = 0
            first = True
            pending_gemm2 = None
            for (e, r0, n_e) in segments:
                w1sb = w1p.tile([P, KO, 2 * FC], dt.bfloat16, tag="w1sb")
                w2sb = w2p.tile([P, K2, H], dt.bfloat16, tag="w2sb")
                if first:
                    # prologue: paired k-slices, x alternating scalar/gpsimd
                    # parallel to w1 pairs on sync — few descriptors (each
                    # costs ~1us of engine issue time) but still progressive
                    # arrival for the k-major loop below.
                    nch0 = _split(n_e)[0]
                    xsb0 = xp.tile([P, KO * NCH], dt.bfloat16, tag="xsb")
                    # paired k-slices, x alternating scalar/gpsimd parallel
                    # to w1 pairs on sync — few descriptors (each costs ~1us
                    # of engine issue time) but still progressive arrival
                    # for the k-major loop below.
                    for i, (ka, kb) in enumerate([(0, 2), (2, 4), (4, 6), (6, 8)]):
                        xq = nc.scalar if i % 2 == 0 else nc.gpsimd
                        xq.dma_start(
                            xsb0[:, ka * nch0 : kb * nch0],
                            xT[:, ka * nch0 : kb * nch0],
                        )
                        nc.sync.dma_start(
                            w1sb[:, ka:kb, :], w1[e, :, ka:kb, :]
                        )
                else:
                    # single-shot per expert: weight prefetch is limited by
                    # DMA issue rate, not bandwidth — 1 descriptor, not 8.
                    xsb0 = None
                    nc.sync.dma_start(w1sb[:], w1[e])
                nc.sync.dma_start(w2sb[:], w2[e])

                for part in _split(n_e):
                    nch = part

                    if xsb0 is not None:
                        xsb, xsb0 = xsb0, None
                    else:
                        xsb = xp.tile([P, KO * NCH], dt.bfloat16, tag="xsb")
                        nc.gpsimd.dma_start(
                            xsb[:, : KO * nch],
                            xT[:, offs[chunk_idx] : offs[chunk_idx] + KO * nch],
                        )

                    def xk(k, nch=nch, xsb=xsb):
                        return xsb[:, k * nch : k * nch + nch]

                    a_tiles = [
                        apool.tile([P, NCH], dt.bfloat16, tag=f"a{j}", name=f"a{j}")
                        if j < 3
                        else a3_tiles[chunk_idx % 2]
                        for j in range(4)
                    ]

                    if first:
                        # k-major: all 7 units accumulate per arriving
                        # (x[k], w1[k]) slice pair — 7 matmuls per DMA step
                        # keeps the PE busy through the cold prologue. Units
                        # 0..5 in hps banks, unit 6 borrows an ops bank
                        # (gemm2 doesn't need it until a chunk later).
                        first = False
                        hu_tiles = [
                            hps.tile([P, NCH], dt.float32, tag="h", name=f"h{u}")
                            for u in range(6)
                        ] + [ops.tile([P, NCH], dt.float32, tag="o", name="h6")]
                        for k in range(KO):
                            for u in range(7):
                                nc.tensor.matmul(
                                    hu_tiles[u][:, :nch],
                                    w1sb[:, k, P * u : P * u + P],
                                    xk(k),
                                    start=(k == 0),
                                    stop=(k == KO - 1),
                                )
                        for u in range(7):
                            act_mul(u, hu_tiles[u], a_tiles, nch)
                    else:
                        # u-major: one PSUM group at a time; silu/mul of
                        # group u overlaps the matmuls of group u+1.
                        for u in (6, 0, 1, 2, 3, 4, 5):
                            hu_ps = hps.tile([P, NCH], dt.float32, tag="h", name=f"h{u}")
                            for k in range(KO):
                                nc.tensor.matmul(
                                    hu_ps[:, :nch],
                                    w1sb[:, k, P * u : P * u + P],
                                    xk(k),
                                    start=(k == 0),
                                    stop=(k == KO - 1),
                                )
                            act_mul(u, hu_ps, a_tiles, nch)

                    # gemm2, TRANSPOSED (emitted one chunk behind gemm1 so
                    # the PE never waits on this chunk's silu/mul chain):
                    # out.T[h, j] = sum_f w2[f, h] * a[f, j]. The moving
                    # operand is the chunk's rows, so a partial chunk
                    # streams exactly nch columns per (h, k) instead of
                    # full 512-col H-slices per 128-row s-tile — ragged
                    # chunks stop paying for padding (FWL keeps the weight
                    # load ~53ns, under the smallest chunk's stream time).
                    def gemm2(
                        ci=chunk_idx, nch=nch, a_tiles=a_tiles, w2sb=w2sb
                    ):
                        osb = opool.tile(
                            [P, KO * NCH], dt.bfloat16, tag="osb", name="osb"
                        )
                        for h in range(H // P):
                            ot = ops.tile(
                                [P, NCH], dt.float32, tag="o", name=f"o{h}"
                            )
                            for k in range(K2):
                                nc.tensor.matmul(
                                    ot[:, :nch],
                                    w2sb[:, k, P * h : P * h + P],
                                    a_tiles[k][:, :nch],
                                    start=(k == 0),
                                    stop=(k == K2 - 1),
                                )
                            nc.vector.tensor_copy(
                                osb[:, h * nch : h * nch + nch], ot[:, :nch]
                            )
                        # one tight contiguous store per chunk (transposed
                        # [h, j] layout; the host transposes back). Alternate
                        # queues to spread descriptor-generation cost.
                        sq = nc.scalar if ci % 2 == 0 else nc.gpsimd
                        sq.dma_start(
                            outp[:, offs[ci] : offs[ci] + KO * nch],
                            osb[:, : KO * nch],
                        )

                    if pending_gemm2 is not None:
                        pending_gemm2()
                    pending_gemm2 = gemm2
                    chunk_idx += 1
            pending_gemm2()

    nc.compile()
    return nc


def _prepare_inputs(hidden_states, w1, w2, chunks):
    """Host-side shard/layout/cast. Returns (xT, [w1c per core], [w2c per core])."""
    x = np.asarray(hidden_states, dtype=np.float32)
    w1 = np.asarray(w1, dtype=np.float32)
    w2 = np.asarray(w2, dtype=np.float32)

    xb = x.astype(BF16)          # [R, H]
    w1b = w1.astype(BF16)        # [E, H, 2F]
    w2b = w2.astype(BF16)        # [E, F, H]

    # tight chunk-major x: for chunk c at column offset off_c,
    # xT[p, off_c + k*nch + j] = x[r_c + j, 128*k + p]
    tot = sum((H // P) * nch for (_, _, nch) in chunks)
    xT = np.empty((P, tot), dtype=BF16)
    off = 0
    for (_, r, nch) in chunks:
        blk = xb[r : r + nch, :].T.reshape(H // P, P, nch)  # [k, p, j]
        xT[:, off : off + (H // P) * nch] = np.ascontiguousarray(
            blk.transpose(1, 0, 2)
        ).reshape(P, (H // P) * nch)
        off += (H // P) * nch

    w1cs, w2cs = [], []
    for c in range(8):
        gate = w1b[:, :, c * FC : (c + 1) * FC]
        up = w1b[:, :, F + c * FC : F + (c + 1) * FC]
        # interleave 64-channel blocks: [G0|U0|G1|U1|...|G6|U6] so each
        # 128-column m-slice u packs gate_u in psum partitions 0:64 and
        # up_u in 64:128.
        w1cat = np.ascontiguousarray(
            np.stack(
                [gate.reshape(E, H, FC // 64, 64), up.reshape(E, H, FC // 64, 64)],
                axis=3,
            ).reshape(E, H, 2 * FC)
        )
        w1c = np.ascontiguousarray(
            w1cat.reshape(E, H // P, P, 2 * FC).transpose(0, 2, 1, 3)
        )
        w2pad = np.zeros((E, FCP, H), dtype=BF16)
        w2pad[:, :FC, :] = w2b[:, c * FC : (c + 1) * FC, :]
        w2c = np.ascontiguousarray(
            w2pad.reshape(E, FCP // P, P, H).transpose(0, 2, 1, 3)
        )
        w1cs.append(w1c)
        w2cs.append(w2c)
    return xT, w1cs, w2cs


def kernel(hidden_states, w1, w2, rows_for_experts):
    global LAST_RESULT
    from concourse.bass_utils import run_bass_kernel_spmd

    segs = _segments(np.asarray(rows_for_experts))
    if not segs:
        return np.zeros((R, H), dtype=np.float32)
    key = tuple(segs)
    nc = _PROGRAM_CACHE.get(key)
    if nc is None:
        nc = _build_program(segs)
        _PROGRAM_CACHE[key] = nc

    chunks = _chunk_list(segs)
    xT, w1cs, w2cs = _prepare_inputs(hidden_states, w1, w2, chunks)
    in_maps = [
        {"xT": xT, "w1c": w1cs[c], "w2c": w2cs[c]} for c in range(8)
    ]
    res = run_bass_kernel_spmd(nc, in_maps, core_ids=list(range(8)))
    LAST_RESULT = res

    acc = np.zeros((R, H), dtype=np.float32)
    for c in range(8):
        blocks = res.results[c]["outp"]  # [P, sum(8*nch)] bf16, transposed
        off = 0
        for (_, r, nch) in chunks:
            blk = blocks[:, off : off + (H // P) * nch].reshape(P, H // P, nch)
            rowsmaj = blk.transpose(1, 0, 2).reshape(H, nch).T
            acc[r : r + nch] += rowsmaj.astype(np.float32)
            off += (H // P) * nch
    return acc
